# revision 53
# baseline (speedup 1.0000x reference)
"""Trainium2 Bass kernel for nn_DTFormer (histogram_binning).

Math: for each batch row and each of src/dst lists, count (id,snap)
multiset matches (self and cross), then run the counts through two tiny
MLPs.  Since the MLP output depends only on (self_count, cross_count,
snap) -- integers with tiny range -- the whole MLP pipeline is
precomputed host-side into a lookup table T[32*32*8, 128] from the
params.  The device kernel does the match counting and a row-gather of
T, data-parallel over the 64 batch rows across 8 cores.

Inputs are packed host-side into a combined key per element:
v = 8*id + (snap-1) < 16384; equality of v <=> equality of (id, snap).
valid = (v >= 8) (padding ids are 0).

Counting: E_xy[j, k] = [v_x[j] == v_y[k]] processed in 8 j-tiles of
[128 x 1024] per matrix; 4 matrices (ss, sd, ds, dd).  The work is
split across engines: ss + sd run as DVE tensor_scalar compares with
fused row-sum accumulation; ds + dd run on PE + ACT: the PE computes
d = (vh_j-vh_k)^2 + (vl_j-vl_k)^2 from bf16 digit-split operands
(7-bit digits, squares split into exact 256-multiples + remainders so
every product is integer-exact), and ACT computes relu(1-d) with
row-sum accumulation.

Output: keys = clamp(self)*256 + clamp(cross)*8 + (snap-1) gathered
from T via gpsimd dma_gather.  Scheduling notes (all load-bearing):
gathers rotate SWDGE queues 1..3 (num_swdge_queues=4) so consecutive
gathers pipeline instead of serializing on one ring; per-list key
chains keep src keys DVE-local and dst keys ACT-local so each fires
as soon as its engine finishes a row; input loads go on the scalar
HWDGE queue so the sync queue carries only the key/idx plumbing
(whose key-waits would otherwise head-of-line block loads and inflate
the gathers' DMA-ring completion thresholds).
"""

import sys

for p in ("/opt/trn_rl_repo", "/root/.axon_site/_ro/trn_rl_repo"):
    if p not in sys.path:
        sys.path.insert(0, p)

import numpy as np
from contextlib import ExitStack

import concourse.bass as bass
import concourse.bacc as bacc
import concourse.tile as tile
from concourse import mybir
from concourse.bass_utils import run_bass_kernel_spmd

B, L, S, D = 64, 1024, 8, 128
NCORES = 8
BPC = B // NCORES          # batches per core
NT = L // 128              # j-tiles per row
CMAX = 32                  # count clamp (counts are tiny; 32 is ample)
TROWS = CMAX * CMAX * S    # 8192 table rows

F32 = mybir.dt.float32
BF16 = mybir.dt.bfloat16
I16 = mybir.dt.int16
ALU = mybir.AluOpType
ACTF = mybir.ActivationFunctionType

_NC_CACHE = {}
TRACE = False
LAST_RESULTS = {}


def build_table(agg_w1, agg_b1, agg_w2, agg_b2, enc_w1, enc_b1, enc_w2, enc_b2):
    """T[a*CMAX*S + b*S + s] = output row for (self=a, cross=b, snap=s+1)."""
    a = np.arange(CMAX, dtype=np.float64)
    w1 = agg_w1.astype(np.float64)      # [S, D]
    b1 = agg_b1.astype(np.float64)      # [D]
    ha = np.maximum(a[None, :, None] * w1[:, None, :] + b1, 0.0)  # [S, CMAX, D]
    g = 0.5 * (ha[:, :, None, :] + ha[:, None, :, :])             # [S, A, B, D]
    y = g @ agg_w2.astype(np.float64) + agg_b2.astype(np.float64)  # [S, A, B, 2]
    ew1 = enc_w1.astype(np.float64)[0]   # [D]
    eb1 = enc_b1.astype(np.float64)
    h0 = np.maximum(y[..., 0:1] * ew1 + eb1, 0.0)  # [S, A, B, D]
    h1 = np.maximum(y[..., 1:2] * ew1 + eb1, 0.0)
    out = (h0 + h1) @ enc_w2.astype(np.float64) + 2.0 * enc_b2.astype(np.float64)
    out = np.transpose(out, (1, 2, 0, 3)).reshape(TROWS, D)  # [A,B,S,D] flat
    return np.ascontiguousarray(out.astype(np.float32))


def _replicate_ap(row_ap, parts=128):
    """AP that reads a DRAM row [N] replicated across `parts` partitions."""
    return bass.AP(tensor=row_ap.tensor, offset=row_ap.offset,
                   ap=[[0, parts]] + [list(p) for p in row_ap.ap])


def build_nc():
    nc = bacc.Bacc("TRN2", num_swdge_queues=4)
    vsn_d = nc.dram_tensor("vsn", [BPC, 128, 2, 2, NT], F32, kind="ExternalInput")
    # bf16 digit-split operands for the PE distance matmuls:
    # block 0: j-side dst, block 1: k-side src, block 2: k-side dst.
    quint_d = nc.dram_tensor("quint_d", [BPC, 8, 3 * L], BF16, kind="ExternalInput")
    v_i = {x: nc.dram_tensor(f"vi_{x}", [BPC, L], I16, kind="ExternalInput")
           for x in ("s", "d")}
    table = nc.dram_tensor("table", [TROWS, D], F32, kind="ExternalInput")
    feat_t = {"s": nc.dram_tensor("src_feat", [BPC, L, D], F32, kind="ExternalOutput"),
              "d": nc.dram_tensor("dst_feat", [BPC, L, D], F32, kind="ExternalOutput")}

    with tile.TileContext(nc) as tc, ExitStack() as ctx:
        small = ctx.enter_context(tc.tile_pool(name="small", bufs=6))
        bcp = ctx.enter_context(tc.tile_pool(name="bcp", bufs=4))
        qp = ctx.enter_context(tc.tile_pool(name="qp", bufs=4))
        pbc = ctx.enter_context(tc.tile_pool(name="pbc", bufs=3, space="PSUM"))
        scr = ctx.enter_context(tc.tile_pool(name="scr", bufs=6))
        feat = ctx.enter_context(tc.tile_pool(name="feat", bufs=6))
        idxp = ctx.enter_context(tc.tile_pool(name="idxp", bufs=3))
        drsc = ctx.enter_context(tc.tile_pool(name="drsc", bufs=4, space="DRAM"))

        gq = [0]  # rotating SWDGE queue index
        pend = []

        def flush(pend_group):
            # baseline-proven idx plumbing: keys roundtrip through DRAM into
            # the wrapped/replicated layout dma_gather expects, with the
            # wrap DMAs shared across the group's lists.
            n = len(pend_group)
            if True:
                # idx plumbing stays on the gpsimd SWDGE queue: its
                # completion semaphores are engine-local and precise, while
                # sync-queue HWDGE completions are tracked by shared
                # per-ring descriptor counters whose thresholds aggregate
                # unrelated loads (observed: first gather stuck behind
                # DMAHW*>=96, i.e. the whole load stream).
                k_scr = drsc.tile([1, n * L], I16, tag="kscr", name="k_scr")
                for q, (b_, x_, kt) in enumerate(pend_group):
                    nc.sync.dma_start(
                        out=k_scr[0, q * L:(q + 1) * L].rearrange(
                            "(p t) -> p t", t=NT),
                        in_=kt[:])
                idxs_sb = idxp.tile([128, n, L // 16], I16, tag="idxs",
                                    name="idxs_sb")
                wrap_ap = k_scr[0, :].rearrange("(q i w) -> i q w", i=16,
                                                w=L // 16)
                for g in range(8):
                    nc.sync.dma_start(
                        out=idxs_sb[16 * g:16 * (g + 1), :, :], in_=wrap_ap)
                for q, (b_, x_, kt) in enumerate(pend_group):
                    ft = feat.tile([128, NT, D], F32, tag="ft", name="ft")
                    # gathers rotate SWDGE queues 1..3; queue 0 is the
                    # mainline gpsimd.dma_start ring used by the stores
                    nc.gpsimd.dma_gather(
                        out_ap=ft[:], in_ap=table[:],
                        idxs_ap=idxs_sb[:, q, :],
                        num_idxs=L, num_idxs_reg=L, elem_size=D,
                        queue_num=1 + gq[0])
                    gq[0] = (gq[0] + 1) % 3
                    nc.gpsimd.dma_start(
                        out=feat_t[x_][b_, :, :].rearrange(
                            "(p q) d -> p q d", q=NT),
                        in_=ft[:])

        for b in range(BPC):
            # loads live on the scalar HWDGE queue so the sync queue
            # carries ONLY the key plumbing: a flush's key-wait must not
            # head-of-line block later rows' loads, and the gathers'
            # ring-counter thresholds must not aggregate the load stream
            vsn_t = small.tile([128, 2, 2, NT], F32, tag="vsn", name="vsn_t")
            nc.scalar.dma_start(out=vsn_t[:], in_=vsn_d[b])
            q5 = qp.tile([8, 3 * L], BF16, tag="q5", name="q5")
            nc.scalar.dma_start(out=q5[:], in_=quint_d[b])
            vb = {}
            for x in ("s", "d"):
                vbx = bcp.tile([128, L], I16, tag="vb" + x, name="vb")
                nc.scalar.dma_start(out=vbx[:], in_=_replicate_ap(v_i[x][b, :]))
                vb[x] = vbx
            # flush the previous 2-row group after this row's loads
            if pend and b % 2 == 0:
                flush(pend)
                pend = []
            vv = {x: vsn_t[:, 0, xi, :] for xi, x in enumerate(("s", "d"))}
            sn2 = vsn_t[:, 1, :, :]
            valid2 = small.tile([128, 2, NT], F32, tag="valid", name="valid2")
            nc.vector.tensor_scalar(
                out=valid2[:], in0=vsn_t[:, 0, :, :], scalar1=8.0, scalar2=None,
                op0=ALU.is_ge)

            # ---- counting ----
            # cnt_s = src-list counts [self|cross] (both produced on DVE),
            # cnt_d = dst-list counts [self|cross] (both produced on ACT),
            # so each list's key chain depends on a single engine.
            cnt_s = small.tile([128, 2, NT], F32, tag="cnt_s", name="cnt_s")
            cnt_d = small.tile([128, 2, NT], F32, tag="cnt_d", name="cnt_d")
            cnt = {"ss": cnt_s[:, 0, :], "sd": cnt_s[:, 1, :],
                   "dd": cnt_d[:, 0, :], "ds": cnt_d[:, 1, :]}
            cnt2 = {"s": cnt_s, "d": cnt_d}
            # DVE: ss and sd (compare against per-partition src scalars).
            # The sd compare outputs E_sd tiles, whose COLUMN sums are the
            # ds counts: the PE accumulates them with a ones-matmul into a
            # [1, L] PSUM row, so the old ds distance+relu path is gone.
            # DVE: ss and sd (compare against per-partition src scalars)
            for t in range(NT):
                o = scr.tile([128, L], BF16, tag="scr_ss", name="o")
                nc.vector.tensor_scalar(
                    out=o[:], in0=vb["s"][:], scalar1=vv["s"][:, t:t + 1],
                    scalar2=0.0, op0=ALU.is_equal, op1=ALU.add,
                    accum_out=cnt["ss"][:, t:t + 1])
                osd = scr.tile([128, L], BF16, tag="scr_sd", name="osd")
                nc.vector.tensor_scalar(
                    out=osd[:], in0=vb["d"][:], scalar1=vv["s"][:, t:t + 1],
                    scalar2=0.0, op0=ALU.is_equal, op1=ALU.add,
                    accum_out=cnt["sd"][:, t:t + 1])
            # PE+ACT: ds (j-side dst vs k-side src) and dd (dst vs dst)
            for m, koff in (("ds", L), ("dd", 2 * L)):
                for t in range(NT):
                    d_ps = pbc.tile([128, L], F32, space="PSUM", tag="dps",
                                    name="d_ps")
                    for h in range(2):
                        nc.tensor.matmul(
                            out=d_ps[:, h * 512:(h + 1) * 512],
                            lhsT=q5[:, t * 128:(t + 1) * 128],
                            rhs=q5[:, koff + h * 512:koff + (h + 1) * 512],
                            start=True, stop=True)
                    o2 = scr.tile([128, L], BF16, tag="scr_a", name="o2")
                    nc.scalar.activation(
                        out=o2[:], in_=d_ps[:], func=ACTF.Relu,
                        bias=1.0, scale=-1.0,
                        accum_out=cnt[m][:, t:t + 1])

            # ---- table keys + gathers, per list ----
            # key = a*CMAX*S + b*S + (sn-1); the src chain depends only on
            # DVE counts and the dst chain only on ACT counts, so each fires
            # as soon as its engine finishes the row.
            for q, x in enumerate(("s", "d")):
                # bounded priority bump: ahead of this row's later compares
                # but keeping monotonic order across rows (offset=None would
                # collapse every row's chain to priority 0 and let the
                # scheduler invert flush order)
                with tc.high_priority(offset=55):
                    a2 = small.tile([128, NT], F32, tag="ka" + x, name="a2")
                    nc.vector.tensor_scalar(
                        out=a2[:], in0=cnt2[x][:, 0, :],
                        scalar1=float(CMAX - 1), scalar2=None, op0=ALU.min)
                    nc.vector.tensor_tensor(
                        out=a2[:], in0=a2[:], in1=valid2[:, q, :],
                        op=ALU.mult)
                    b2 = small.tile([128, NT], F32, tag="kb" + x, name="b2")
                    nc.vector.tensor_scalar(
                        out=b2[:], in0=cnt2[x][:, 1, :],
                        scalar1=float(CMAX - 1), scalar2=None, op0=ALU.min)
                    nc.vector.tensor_tensor(
                        out=b2[:], in0=b2[:], in1=valid2[:, q, :],
                        op=ALU.mult)
                    key2 = small.tile([128, NT], F32, tag="key" + x,
                                      name="key2")
                    nc.vector.scalar_tensor_tensor(
                        out=key2[:], in0=a2[:], scalar=float(CMAX * S),
                        in1=sn2[:, q, :], op0=ALU.mult, op1=ALU.add)
                    nc.vector.scalar_tensor_tensor(
                        out=key2[:], in0=b2[:], scalar=float(S), in1=key2[:],
                        op0=ALU.mult, op1=ALU.add)
                    keyi = small.tile([128, NT], I16, tag="keyi" + x,
                                      name="keyi")
                    nc.vector.tensor_copy(out=keyi[:], in_=key2[:])

                pend.append((b, x, keyi))

            # flush the last two rows individually so the end-of-kernel
            # serial chain (keys -> kscr -> wraps -> gather -> store) only
            # covers one row
            if b >= BPC - 2:
                flush(pend)
                pend = []
    nc.compile()
    return nc


def kernel(src_padded_nodes_neighbor_ids, dst_padded_nodes_neighbor_ids,
           src_padded_nodes_snapshots, dst_padded_nodes_snapshots,
           num_snapshots,
           agg_w1, agg_b1, agg_w2, agg_b2, enc_w1, enc_b1, enc_w2, enc_b2):
    import ml_dtypes

    tab = build_table(np.asarray(agg_w1), np.asarray(agg_b1),
                      np.asarray(agg_w2), np.asarray(agg_b2),
                      np.asarray(enc_w1), np.asarray(enc_b1),
                      np.asarray(enc_w2), np.asarray(enc_b2))

    if "nc" not in _NC_CACHE:
        _NC_CACHE["nc"] = build_nc()
    nc = _NC_CACHE["nc"]

    ids = {"s": np.asarray(src_padded_nodes_neighbor_ids).astype(np.int64),
           "d": np.asarray(dst_padded_nodes_neighbor_ids).astype(np.int64)}
    sn = {"s": np.asarray(src_padded_nodes_snapshots).astype(np.int64),
          "d": np.asarray(dst_padded_nodes_snapshots).astype(np.int64)}
    v = {x: ids[x] * 8 + (sn[x] - 1) for x in ("s", "d")}

    def digit_split(vz):
        """j-side and k-side bf16 digit-split operand blocks [8, n]."""
        vh = (vz >> 7).astype(np.float64)
        vl = (vz & 127).astype(np.float64)
        vh2, vl2 = vh * vh, vl * vl
        k2 = vh2 + vl2
        one = np.ones_like(vh)

        def s256(x):
            hi = np.floor(x / 256.0) * 256.0
            return hi, x - hi

        vh2hi, vh2lo = s256(vh2)
        vl2hi, vl2lo = s256(vl2)
        k2hi, k2lo = s256(k2)
        qj = np.stack([vh2hi, vh2lo, vh, vl2hi, vl2lo, vl, one, one], axis=1)
        qk = np.stack([one, one, -2.0 * vh, one, one, -2.0 * vl,
                       k2hi, k2lo], axis=1)
        return qj, qk

    in_maps = []
    for c in range(NCORES):
        sl = slice(c * BPC, (c + 1) * BPC)
        m = {"table": tab}
        # vsn[b, p, c(v/sn), x(s/d), t]
        vs = np.stack([np.stack([v["s"][sl], v["d"][sl]], axis=1),
                       np.stack([sn["s"][sl] - 1, sn["d"][sl] - 1], axis=1)],
                      axis=1).astype(np.float32)          # [BPC, 2, 2, L]
        vs = vs.reshape(-1, 2, 2, NT, 128).transpose(0, 4, 1, 2, 3)
        m["vsn"] = np.ascontiguousarray(vs)
        qj_d, qk_d = digit_split(v["d"][sl])
        _, qk_s = digit_split(v["s"][sl])
        q = np.concatenate([qj_d, qk_s, qk_d], axis=2)    # [BPC, 8, 3L]
        m["quint_d"] = np.ascontiguousarray(q.astype(ml_dtypes.bfloat16))
        for x in ("s", "d"):
            m[f"vi_{x}"] = np.ascontiguousarray(v[x][sl].astype(np.int16))
        in_maps.append(m)
    res = run_bass_kernel_spmd(nc, in_maps, core_ids=list(range(NCORES)),
                               trace=TRACE)
    LAST_RESULTS["res"] = res
    src_feat = np.concatenate([r["src_feat"] for r in res.results], axis=0)
    dst_feat = np.concatenate([r["dst_feat"] for r in res.results], axis=0)
    return (src_feat, dst_feat)


# revision 54
# speedup vs baseline: 1.0372x; 1.0372x over previous
"""Trainium2 Bass kernel for nn_DTFormer (histogram_binning).

Math: for each batch row and each of src/dst lists, count (id,snap)
multiset matches (self and cross), then run the counts through two tiny
MLPs.  Since the MLP output depends only on (self_count, cross_count,
snap) -- integers with tiny range -- the whole MLP pipeline is
precomputed host-side into a lookup table T[32*32*8, 128] from the
params.  The device kernel does the match counting and a row-gather of
T, data-parallel over the 64 batch rows across 8 cores.

Inputs are packed host-side into a combined key per element:
v = 8*id + (snap-1) < 16384; equality of v <=> equality of (id, snap).
valid = (v >= 8) (padding ids are 0).

Counting: E_xy[j, k] = [v_x[j] == v_y[k]] processed in 8 j-tiles of
[128 x 1024] per matrix; 4 matrices (ss, sd, ds, dd).  The work is
split across engines: ss + sd run as DVE tensor_scalar compares with
fused row-sum accumulation; ds + dd run on PE + ACT: the PE computes
d = (vh_j-vh_k)^2 + (vl_j-vl_k)^2 from bf16 digit-split operands
(7-bit digits, squares split into exact 256-multiples + remainders so
every product is integer-exact), and ACT computes relu(1-d) with
row-sum accumulation.

Output: keys = clamp(self)*256 + clamp(cross)*8 + (snap-1) gathered
from T via gpsimd dma_gather.  Scheduling notes (all load-bearing):
gathers rotate SWDGE queues 1..3 (num_swdge_queues=4) so consecutive
gathers pipeline instead of serializing on one ring; per-list key
chains keep src keys DVE-local and dst keys ACT-local so each fires
as soon as its engine finishes a row; input loads go on the scalar
HWDGE queue so the sync queue carries only the key/idx plumbing
(whose key-waits would otherwise head-of-line block loads and inflate
the gathers' DMA-ring completion thresholds).
"""

import sys

for p in ("/opt/trn_rl_repo", "/root/.axon_site/_ro/trn_rl_repo"):
    if p not in sys.path:
        sys.path.insert(0, p)

import numpy as np
from contextlib import ExitStack

import concourse.bass as bass
import concourse.bacc as bacc
import concourse.tile as tile
from concourse import mybir
from concourse.bass_utils import run_bass_kernel_spmd

B, L, S, D = 64, 1024, 8, 128
NCORES = 8
BPC = B // NCORES          # batches per core
NT = L // 128              # j-tiles per row
CMAX = 32                  # count clamp (counts are tiny; 32 is ample)
TROWS = CMAX * CMAX * S    # 8192 table rows

F32 = mybir.dt.float32
BF16 = mybir.dt.bfloat16
I16 = mybir.dt.int16
ALU = mybir.AluOpType
ACTF = mybir.ActivationFunctionType

_NC_CACHE = {}
TRACE = False
LAST_RESULTS = {}


def build_table(agg_w1, agg_b1, agg_w2, agg_b2, enc_w1, enc_b1, enc_w2, enc_b2):
    """T[a*CMAX*S + b*S + s] = output row for (self=a, cross=b, snap=s+1)."""
    a = np.arange(CMAX, dtype=np.float64)
    w1 = agg_w1.astype(np.float64)      # [S, D]
    b1 = agg_b1.astype(np.float64)      # [D]
    ha = np.maximum(a[None, :, None] * w1[:, None, :] + b1, 0.0)  # [S, CMAX, D]
    g = 0.5 * (ha[:, :, None, :] + ha[:, None, :, :])             # [S, A, B, D]
    y = g @ agg_w2.astype(np.float64) + agg_b2.astype(np.float64)  # [S, A, B, 2]
    ew1 = enc_w1.astype(np.float64)[0]   # [D]
    eb1 = enc_b1.astype(np.float64)
    h0 = np.maximum(y[..., 0:1] * ew1 + eb1, 0.0)  # [S, A, B, D]
    h1 = np.maximum(y[..., 1:2] * ew1 + eb1, 0.0)
    out = (h0 + h1) @ enc_w2.astype(np.float64) + 2.0 * enc_b2.astype(np.float64)
    out = np.transpose(out, (1, 2, 0, 3)).reshape(TROWS, D)  # [A,B,S,D] flat
    return np.ascontiguousarray(out.astype(np.float32))


def _replicate_ap(row_ap, parts=128):
    """AP that reads a DRAM row [N] replicated across `parts` partitions."""
    return bass.AP(tensor=row_ap.tensor, offset=row_ap.offset,
                   ap=[[0, parts]] + [list(p) for p in row_ap.ap])


def build_nc():
    nc = bacc.Bacc("TRN2", num_swdge_queues=4)
    vsn_d = nc.dram_tensor("vsn", [BPC, 128, 2, 2, NT], F32, kind="ExternalInput")
    # bf16 digit-split operands for the PE distance matmuls:
    # block 0: j-side dst, block 1: k-side src, block 2: k-side dst.
    quint_d = nc.dram_tensor("quint_d", [BPC, 8, 4 * L], BF16, kind="ExternalInput")
    v_i = {x: nc.dram_tensor(f"vi_{x}", [BPC, L], I16, kind="ExternalInput")
           for x in ("s", "d")}
    table = nc.dram_tensor("table", [TROWS, D], F32, kind="ExternalInput")
    feat_t = {"s": nc.dram_tensor("src_feat", [BPC, L, D], F32, kind="ExternalOutput"),
              "d": nc.dram_tensor("dst_feat", [BPC, L, D], F32, kind="ExternalOutput")}

    with tile.TileContext(nc) as tc, ExitStack() as ctx:
        small = ctx.enter_context(tc.tile_pool(name="small", bufs=6))
        bcp = ctx.enter_context(tc.tile_pool(name="bcp", bufs=4))
        qp = ctx.enter_context(tc.tile_pool(name="qp", bufs=4))
        pbc = ctx.enter_context(tc.tile_pool(name="pbc", bufs=3, space="PSUM"))
        scr = ctx.enter_context(tc.tile_pool(name="scr", bufs=6))
        feat = ctx.enter_context(tc.tile_pool(name="feat", bufs=6))
        idxp = ctx.enter_context(tc.tile_pool(name="idxp", bufs=3))
        drsc = ctx.enter_context(tc.tile_pool(name="drsc", bufs=4, space="DRAM"))

        gq = [0]  # rotating SWDGE queue index
        pend = []

        def flush(pend_group):
            # baseline-proven idx plumbing: keys roundtrip through DRAM into
            # the wrapped/replicated layout dma_gather expects, with the
            # wrap DMAs shared across the group's lists.
            n = len(pend_group)
            if True:
                # idx plumbing stays on the gpsimd SWDGE queue: its
                # completion semaphores are engine-local and precise, while
                # sync-queue HWDGE completions are tracked by shared
                # per-ring descriptor counters whose thresholds aggregate
                # unrelated loads (observed: first gather stuck behind
                # DMAHW*>=96, i.e. the whole load stream).
                k_scr = drsc.tile([1, n * L], I16, tag="kscr", name="k_scr")
                for q, (b_, x_, kt) in enumerate(pend_group):
                    nc.sync.dma_start(
                        out=k_scr[0, q * L:(q + 1) * L].rearrange(
                            "(p t) -> p t", t=NT),
                        in_=kt)
                idxs_sb = idxp.tile([128, n, L // 16], I16, tag="idxs",
                                    name="idxs_sb")
                wrap_ap = k_scr[0, :].rearrange("(q i w) -> i q w", i=16,
                                                w=L // 16)
                for g in range(8):
                    nc.sync.dma_start(
                        out=idxs_sb[16 * g:16 * (g + 1), :, :], in_=wrap_ap)
                for q, (b_, x_, kt) in enumerate(pend_group):
                    ft = feat.tile([128, NT, D], F32, tag="ft", name="ft")
                    # gathers rotate SWDGE queues 1..3; queue 0 is the
                    # mainline gpsimd.dma_start ring used by the stores
                    nc.gpsimd.dma_gather(
                        out_ap=ft[:], in_ap=table[:],
                        idxs_ap=idxs_sb[:, q, :],
                        num_idxs=L, num_idxs_reg=L, elem_size=D,
                        queue_num=1 + gq[0])
                    gq[0] = (gq[0] + 1) % 3
                    nc.gpsimd.dma_start(
                        out=feat_t[x_][b_, :, :].rearrange(
                            "(p q) d -> p q d", q=NT),
                        in_=ft[:])

        for b in range(BPC):
            # loads live on the scalar HWDGE queue so the sync queue
            # carries ONLY the key plumbing: a flush's key-wait must not
            # head-of-line block later rows' loads, and the gathers'
            # ring-counter thresholds must not aggregate the load stream
            vsn_t = small.tile([128, 2, 2, NT], F32, tag="vsn", name="vsn_t")
            nc.scalar.dma_start(out=vsn_t[:], in_=vsn_d[b])
            q5 = qp.tile([8, 4 * L], BF16, tag="q5", name="q5")
            nc.scalar.dma_start(out=q5[:], in_=quint_d[b])
            vb = {}
            for x in ("s", "d"):
                vbx = bcp.tile([128, L], I16, tag="vb" + x, name="vb")
                nc.scalar.dma_start(out=vbx[:], in_=_replicate_ap(v_i[x][b, :]))
                vb[x] = vbx
            # flush the previous 2-row group after this row's loads
            if pend and b % 2 == 0:
                flush(pend)
                pend = []
            vv = {x: vsn_t[:, 0, xi, :] for xi, x in enumerate(("s", "d"))}
            sn2 = vsn_t[:, 1, :, :]
            valid2 = small.tile([128, 2, NT], F32, tag="valid", name="valid2")
            nc.vector.tensor_scalar(
                out=valid2[:], in0=vsn_t[:, 0, :, :], scalar1=8.0, scalar2=None,
                op0=ALU.is_ge)

            # ---- counting ----
            # cnt_a = self counts [ss | dd], cnt_b = cross counts [sd | ds]
            # (aligned with the key math's [src | dst] list axis)
            cnt_a = small.tile([128, 2, NT], F32, tag="cnt_a", name="cnt_a")
            cnt_b = small.tile([128, 2, NT], F32, tag="cnt_b", name="cnt_b")
            cnt = {"ss": cnt_a[:, 0, :], "dd": cnt_a[:, 1, :],
                   "sd": cnt_b[:, 0, :], "ds": cnt_b[:, 1, :]}
            # DVE: ss and sd (compare against per-partition src scalars).
            # The sd compare outputs E_sd tiles, whose COLUMN sums are the
            # ds counts: the PE accumulates them with a ones-matmul into a
            # [1, L] PSUM row, so the old ds distance+relu path is gone.
            # DVE: ss (t>=1) and sd; ss t=0 is shifted to PE+ACT to
            # balance the engines (DVE also carries the key chains)
            for t in range(NT):
                if t >= 1:
                    o = scr.tile([128, L], BF16, tag="scr_ss", name="o")
                    nc.vector.tensor_scalar(
                        out=o[:], in0=vb["s"][:], scalar1=vv["s"][:, t:t + 1],
                        scalar2=0.0, op0=ALU.is_equal, op1=ALU.add,
                        accum_out=cnt["ss"][:, t:t + 1])
                osd = scr.tile([128, L], BF16, tag="scr_sd", name="osd")
                nc.vector.tensor_scalar(
                    out=osd[:], in0=vb["d"][:], scalar1=vv["s"][:, t:t + 1],
                    scalar2=0.0, op0=ALU.is_equal, op1=ALU.add,
                    accum_out=cnt["sd"][:, t:t + 1])
            # PE+ACT: shifted ss tile, then ds (j dst vs k src), dd
            for m, jo, koff, ts in (("ss", 3 * L, L, (0,)),
                                    ("ds", 0, L, range(NT)),
                                    ("dd", 0, 2 * L, range(NT))):
                for t in ts:
                    d_ps = pbc.tile([128, L], F32, space="PSUM", tag="dps",
                                    name="d_ps")
                    for h in range(2):
                        nc.tensor.matmul(
                            out=d_ps[:, h * 512:(h + 1) * 512],
                            lhsT=q5[:, jo + t * 128:jo + (t + 1) * 128],
                            rhs=q5[:, koff + h * 512:koff + (h + 1) * 512],
                            start=True, stop=True)
                    o2 = scr.tile([128, L], BF16, tag="scr_a", name="o2")
                    nc.scalar.activation(
                        out=o2[:], in_=d_ps[:], func=ACTF.Relu,
                        bias=1.0, scale=-1.0,
                        accum_out=cnt[m][:, t:t + 1])

            # ---- table keys, both lists in one chain ----
            # key = a*CMAX*S + b*S + (sn-1).  ACT rows complete before DVE
            # rows in steady state, so a single [128, 2, NT] chain (7 DVE
            # ops instead of 14) costs no waiting.
            with tc.high_priority(offset=55):
                a2 = small.tile([128, 2, NT], F32, tag="ka", name="a2")
                nc.vector.tensor_scalar(
                    out=a2[:], in0=cnt_a[:],
                    scalar1=float(CMAX - 1), scalar2=None, op0=ALU.min)
                nc.vector.tensor_tensor(
                    out=a2[:], in0=a2[:], in1=valid2[:], op=ALU.mult)
                b2 = small.tile([128, 2, NT], F32, tag="kb", name="b2")
                nc.vector.tensor_scalar(
                    out=b2[:], in0=cnt_b[:],
                    scalar1=float(CMAX - 1), scalar2=None, op0=ALU.min)
                nc.vector.tensor_tensor(
                    out=b2[:], in0=b2[:], in1=valid2[:], op=ALU.mult)
                key2 = small.tile([128, 2, NT], F32, tag="key", name="key2")
                nc.vector.scalar_tensor_tensor(
                    out=key2[:], in0=a2[:], scalar=float(CMAX * S),
                    in1=sn2[:], op0=ALU.mult, op1=ALU.add)
                nc.vector.scalar_tensor_tensor(
                    out=key2[:], in0=b2[:], scalar=float(S), in1=key2[:],
                    op0=ALU.mult, op1=ALU.add)
                keyi = small.tile([128, 2, NT], I16, tag="keyi", name="keyi")
                nc.vector.tensor_copy(out=keyi[:], in_=key2[:])

            for q, x in enumerate(("s", "d")):
                pend.append((b, x, keyi[:, q, :]))

            # flush the last two rows individually so the end-of-kernel
            # serial chain (keys -> kscr -> wraps -> gather -> store) only
            # covers one row
            if b >= BPC - 2:
                flush(pend)
                pend = []
    nc.compile()
    return nc


def kernel(src_padded_nodes_neighbor_ids, dst_padded_nodes_neighbor_ids,
           src_padded_nodes_snapshots, dst_padded_nodes_snapshots,
           num_snapshots,
           agg_w1, agg_b1, agg_w2, agg_b2, enc_w1, enc_b1, enc_w2, enc_b2):
    import ml_dtypes

    tab = build_table(np.asarray(agg_w1), np.asarray(agg_b1),
                      np.asarray(agg_w2), np.asarray(agg_b2),
                      np.asarray(enc_w1), np.asarray(enc_b1),
                      np.asarray(enc_w2), np.asarray(enc_b2))

    if "nc" not in _NC_CACHE:
        _NC_CACHE["nc"] = build_nc()
    nc = _NC_CACHE["nc"]

    ids = {"s": np.asarray(src_padded_nodes_neighbor_ids).astype(np.int64),
           "d": np.asarray(dst_padded_nodes_neighbor_ids).astype(np.int64)}
    sn = {"s": np.asarray(src_padded_nodes_snapshots).astype(np.int64),
          "d": np.asarray(dst_padded_nodes_snapshots).astype(np.int64)}
    v = {x: ids[x] * 8 + (sn[x] - 1) for x in ("s", "d")}

    def digit_split(vz):
        """j-side and k-side bf16 digit-split operand blocks [8, n]."""
        vh = (vz >> 7).astype(np.float64)
        vl = (vz & 127).astype(np.float64)
        vh2, vl2 = vh * vh, vl * vl
        k2 = vh2 + vl2
        one = np.ones_like(vh)

        def s256(x):
            hi = np.floor(x / 256.0) * 256.0
            return hi, x - hi

        vh2hi, vh2lo = s256(vh2)
        vl2hi, vl2lo = s256(vl2)
        k2hi, k2lo = s256(k2)
        qj = np.stack([vh2hi, vh2lo, vh, vl2hi, vl2lo, vl, one, one], axis=1)
        qk = np.stack([one, one, -2.0 * vh, one, one, -2.0 * vl,
                       k2hi, k2lo], axis=1)
        return qj, qk

    in_maps = []
    for c in range(NCORES):
        sl = slice(c * BPC, (c + 1) * BPC)
        m = {"table": tab}
        # vsn[b, p, c(v/sn), x(s/d), t]
        vs = np.stack([np.stack([v["s"][sl], v["d"][sl]], axis=1),
                       np.stack([sn["s"][sl] - 1, sn["d"][sl] - 1], axis=1)],
                      axis=1).astype(np.float32)          # [BPC, 2, 2, L]
        vs = vs.reshape(-1, 2, 2, NT, 128).transpose(0, 4, 1, 2, 3)
        m["vsn"] = np.ascontiguousarray(vs)
        qj_d, qk_d = digit_split(v["d"][sl])
        qj_s, qk_s = digit_split(v["s"][sl])
        q = np.concatenate([qj_d, qk_s, qk_d, qj_s], axis=2)  # [BPC, 8, 4L]
        m["quint_d"] = np.ascontiguousarray(q.astype(ml_dtypes.bfloat16))
        for x in ("s", "d"):
            m[f"vi_{x}"] = np.ascontiguousarray(v[x][sl].astype(np.int16))
        in_maps.append(m)
    res = run_bass_kernel_spmd(nc, in_maps, core_ids=list(range(NCORES)),
                               trace=TRACE)
    LAST_RESULTS["res"] = res
    src_feat = np.concatenate([r["src_feat"] for r in res.results], axis=0)
    dst_feat = np.concatenate([r["dst_feat"] for r in res.results], axis=0)
    return (src_feat, dst_feat)


# revision 55
# speedup vs baseline: 1.0650x; 1.0269x over previous
"""Trainium2 Bass kernel for nn_DTFormer (histogram_binning).

Math: for each batch row and each of src/dst lists, count (id,snap)
multiset matches (self and cross), then run the counts through two tiny
MLPs.  Since the MLP output depends only on (self_count, cross_count,
snap) -- integers with tiny range -- the whole MLP pipeline is
precomputed host-side into a lookup table T[32*32*8, 128] from the
params.  The device kernel does the match counting and a row-gather of
T, data-parallel over the 64 batch rows across 8 cores.

Inputs are packed host-side into a combined key per element:
v = 8*id + (snap-1) < 16384; equality of v <=> equality of (id, snap).
valid = (v >= 8) (padding ids are 0).

Counting: E_xy[j, k] = [v_x[j] == v_y[k]] processed in 8 j-tiles of
[128 x 1024] per matrix; 4 matrices (ss, sd, ds, dd).  The work is
split across engines: ss + sd run as DVE tensor_scalar compares with
fused row-sum accumulation; ds + dd run on PE + ACT: the PE computes
d = (vh_j-vh_k)^2 + (vl_j-vl_k)^2 from bf16 digit-split operands
(7-bit digits, squares split into exact 256-multiples + remainders so
every product is integer-exact), and ACT computes relu(1-d) with
row-sum accumulation.

Output: keys = clamp(self)*256 + clamp(cross)*8 + (snap-1) gathered
from T via gpsimd dma_gather.  Scheduling notes (all load-bearing):
gathers rotate SWDGE queues 1..3 (num_swdge_queues=4) so consecutive
gathers pipeline instead of serializing on one ring; per-list key
chains keep src keys DVE-local and dst keys ACT-local so each fires
as soon as its engine finishes a row; input loads go on the scalar
HWDGE queue so the sync queue carries only the key/idx plumbing
(whose key-waits would otherwise head-of-line block loads and inflate
the gathers' DMA-ring completion thresholds).
"""

import sys

for p in ("/opt/trn_rl_repo", "/root/.axon_site/_ro/trn_rl_repo"):
    if p not in sys.path:
        sys.path.insert(0, p)

import numpy as np
from contextlib import ExitStack

import concourse.bass as bass
import concourse.bacc as bacc
import concourse.tile as tile
from concourse import mybir
from concourse.bass_utils import run_bass_kernel_spmd

B, L, S, D = 64, 1024, 8, 128
NCORES = 8
BPC = B // NCORES          # batches per core
NT = L // 128              # j-tiles per row
CMAX = 32                  # count clamp (counts are tiny; 32 is ample)
TROWS = CMAX * CMAX * S    # 8192 table rows

F32 = mybir.dt.float32
BF16 = mybir.dt.bfloat16
I16 = mybir.dt.int16
ALU = mybir.AluOpType
ACTF = mybir.ActivationFunctionType

_NC_CACHE = {}
TRACE = False
LAST_RESULTS = {}


def build_table(agg_w1, agg_b1, agg_w2, agg_b2, enc_w1, enc_b1, enc_w2, enc_b2):
    """T[a*CMAX*S + b*S + s] = output row for (self=a, cross=b, snap=s+1)."""
    a = np.arange(CMAX, dtype=np.float64)
    w1 = agg_w1.astype(np.float64)      # [S, D]
    b1 = agg_b1.astype(np.float64)      # [D]
    ha = np.maximum(a[None, :, None] * w1[:, None, :] + b1, 0.0)  # [S, CMAX, D]
    g = 0.5 * (ha[:, :, None, :] + ha[:, None, :, :])             # [S, A, B, D]
    y = g @ agg_w2.astype(np.float64) + agg_b2.astype(np.float64)  # [S, A, B, 2]
    ew1 = enc_w1.astype(np.float64)[0]   # [D]
    eb1 = enc_b1.astype(np.float64)
    h0 = np.maximum(y[..., 0:1] * ew1 + eb1, 0.0)  # [S, A, B, D]
    h1 = np.maximum(y[..., 1:2] * ew1 + eb1, 0.0)
    out = (h0 + h1) @ enc_w2.astype(np.float64) + 2.0 * enc_b2.astype(np.float64)
    out = np.transpose(out, (1, 2, 0, 3)).reshape(TROWS, D)  # [A,B,S,D] flat
    return np.ascontiguousarray(out.astype(np.float32))


def _replicate_ap(row_ap, parts=128):
    """AP that reads a DRAM row [N] replicated across `parts` partitions."""
    return bass.AP(tensor=row_ap.tensor, offset=row_ap.offset,
                   ap=[[0, parts]] + [list(p) for p in row_ap.ap])


def build_nc():
    nc = bacc.Bacc("TRN2", num_swdge_queues=4)
    vsn_d = nc.dram_tensor("vsn", [BPC, 128, 2, 2, NT], F32, kind="ExternalInput")
    # bf16 digit-split operands for the PE distance matmuls:
    # block 0: j-side dst, block 1: k-side src, block 2: k-side dst.
    quint_d = nc.dram_tensor("quint_d", [BPC, 8, 3 * L], BF16, kind="ExternalInput")
    v_i = {x: nc.dram_tensor(f"vi_{x}", [BPC, L], I16, kind="ExternalInput")
           for x in ("s", "d")}
    table = nc.dram_tensor("table", [TROWS, D], F32, kind="ExternalInput")
    feat_t = {"s": nc.dram_tensor("src_feat", [BPC, L, D], F32, kind="ExternalOutput"),
              "d": nc.dram_tensor("dst_feat", [BPC, L, D], F32, kind="ExternalOutput")}

    with tile.TileContext(nc) as tc, ExitStack() as ctx:
        small = ctx.enter_context(tc.tile_pool(name="small", bufs=6))
        bcp = ctx.enter_context(tc.tile_pool(name="bcp", bufs=4))
        qp = ctx.enter_context(tc.tile_pool(name="qp", bufs=4))
        pbc = ctx.enter_context(tc.tile_pool(name="pbc", bufs=3, space="PSUM"))
        scr = ctx.enter_context(tc.tile_pool(name="scr", bufs=6))
        feat = ctx.enter_context(tc.tile_pool(name="feat", bufs=6))
        idxp = ctx.enter_context(tc.tile_pool(name="idxp", bufs=3))
        drsc = ctx.enter_context(tc.tile_pool(name="drsc", bufs=4, space="DRAM"))

        gq = [0]  # rotating SWDGE queue index
        pend = []

        def flush(pend_group):
            # baseline-proven idx plumbing: keys roundtrip through DRAM into
            # the wrapped/replicated layout dma_gather expects, with the
            # wrap DMAs shared across the group's lists.
            n = len(pend_group)
            if True:
                # idx plumbing stays on the gpsimd SWDGE queue: its
                # completion semaphores are engine-local and precise, while
                # sync-queue HWDGE completions are tracked by shared
                # per-ring descriptor counters whose thresholds aggregate
                # unrelated loads (observed: first gather stuck behind
                # DMAHW*>=96, i.e. the whole load stream).
                k_scr = drsc.tile([1, n * L], I16, tag="kscr", name="k_scr")
                for q, (b_, x_, kt) in enumerate(pend_group):
                    nc.sync.dma_start(
                        out=k_scr[0, q * L:(q + 1) * L].rearrange(
                            "(p t) -> p t", t=NT),
                        in_=kt[:])
                idxs_sb = idxp.tile([128, n, L // 16], I16, tag="idxs",
                                    name="idxs_sb")
                wrap_ap = k_scr[0, :].rearrange("(q i w) -> i q w", i=16,
                                                w=L // 16)
                for g in range(8):
                    nc.sync.dma_start(
                        out=idxs_sb[16 * g:16 * (g + 1), :, :], in_=wrap_ap)
                for q, (b_, x_, kt) in enumerate(pend_group):
                    ft = feat.tile([128, NT, D], F32, tag="ft", name="ft")
                    # gathers rotate SWDGE queues 1..3; queue 0 is the
                    # mainline gpsimd.dma_start ring used by the stores
                    nc.gpsimd.dma_gather(
                        out_ap=ft[:], in_ap=table[:],
                        idxs_ap=idxs_sb[:, q, :],
                        num_idxs=L, num_idxs_reg=L, elem_size=D,
                        queue_num=1 + gq[0])
                    gq[0] = (gq[0] + 1) % 3
                    nc.gpsimd.dma_start(
                        out=feat_t[x_][b_, :, :].rearrange(
                            "(p q) d -> p q d", q=NT),
                        in_=ft[:])

        for b in range(BPC):
            # loads live on the scalar HWDGE queue so the sync queue
            # carries ONLY the key plumbing: a flush's key-wait must not
            # head-of-line block later rows' loads, and the gathers'
            # ring-counter thresholds must not aggregate the load stream
            vsn_t = small.tile([128, 2, 2, NT], F32, tag="vsn", name="vsn_t")
            nc.scalar.dma_start(out=vsn_t[:], in_=vsn_d[b])
            q5 = qp.tile([8, 3 * L], BF16, tag="q5", name="q5")
            nc.scalar.dma_start(out=q5[:], in_=quint_d[b])
            vb = {}
            for x in ("s", "d"):
                vbx = bcp.tile([128, L], I16, tag="vb" + x, name="vb")
                nc.scalar.dma_start(out=vbx[:], in_=_replicate_ap(v_i[x][b, :]))
                vb[x] = vbx
            # flush the previous 2-row group after this row's loads
            if pend and b % 2 == 0:
                flush(pend)
                pend = []
            vv = {x: vsn_t[:, 0, xi, :] for xi, x in enumerate(("s", "d"))}
            sn2 = vsn_t[:, 1, :, :]
            valid2 = small.tile([128, 2, NT], F32, tag="valid", name="valid2")
            nc.vector.tensor_scalar(
                out=valid2[:], in0=vsn_t[:, 0, :, :], scalar1=8.0, scalar2=None,
                op0=ALU.is_ge)

            # ---- counting ----
            # cnt_s = src-list counts [self|cross] (both produced on DVE),
            # cnt_d = dst-list counts [self|cross] (both produced on ACT),
            # so each list's key chain depends on a single engine.
            cnt_s = small.tile([128, 2, NT], F32, tag="cnt_s", name="cnt_s")
            cnt_d = small.tile([128, 2, NT], F32, tag="cnt_d", name="cnt_d")
            cnt = {"ss": cnt_s[:, 0, :], "sd": cnt_s[:, 1, :],
                   "dd": cnt_d[:, 0, :], "ds": cnt_d[:, 1, :]}
            cnt2 = {"s": cnt_s, "d": cnt_d}
            # DVE: ss and sd (compare against per-partition src scalars).
            # The sd compare outputs E_sd tiles, whose COLUMN sums are the
            # ds counts: the PE accumulates them with a ones-matmul into a
            # [1, L] PSUM row, so the old ds distance+relu path is gone.
            # DVE: ss and sd (compare against per-partition src scalars)
            for t in range(NT):
                o = scr.tile([128, L], BF16, tag="scr_ss", name="o")
                nc.vector.tensor_scalar(
                    out=o[:], in0=vb["s"][:], scalar1=vv["s"][:, t:t + 1],
                    scalar2=0.0, op0=ALU.is_equal, op1=ALU.add,
                    accum_out=cnt["ss"][:, t:t + 1])
                osd = scr.tile([128, L], BF16, tag="scr_sd", name="osd")
                nc.vector.tensor_scalar(
                    out=osd[:], in0=vb["d"][:], scalar1=vv["s"][:, t:t + 1],
                    scalar2=0.0, op0=ALU.is_equal, op1=ALU.add,
                    accum_out=cnt["sd"][:, t:t + 1])
            # PE+ACT: ds (j-side dst vs k-side src) and dd (dst vs dst)
            for m, koff in (("ds", L), ("dd", 2 * L)):
                for t in range(NT):
                    d_ps = pbc.tile([128, L], F32, space="PSUM", tag="dps",
                                    name="d_ps")
                    for h in range(2):
                        nc.tensor.matmul(
                            out=d_ps[:, h * 512:(h + 1) * 512],
                            lhsT=q5[:, t * 128:(t + 1) * 128],
                            rhs=q5[:, koff + h * 512:koff + (h + 1) * 512],
                            start=True, stop=True)
                    o2 = scr.tile([128, L], BF16, tag="scr_a", name="o2")
                    nc.scalar.activation(
                        out=o2[:], in_=d_ps[:], func=ACTF.Relu,
                        bias=1.0, scale=-1.0,
                        accum_out=cnt[m][:, t:t + 1])

            # ---- table keys + gathers, per list ----
            # key = a*CMAX*S + b*S + (sn-1); the src chain depends only on
            # DVE counts and the dst chain only on ACT counts, so each fires
            # as soon as its engine finishes the row.
            for q, x in enumerate(("s", "d")):
                # bounded priority bump: ahead of this row's later compares
                # but keeping monotonic order across rows (offset=None would
                # collapse every row's chain to priority 0 and let the
                # scheduler invert flush order)
                with tc.high_priority(offset=55):
                    a2 = small.tile([128, NT], F32, tag="ka" + x, name="a2")
                    nc.vector.tensor_scalar(
                        out=a2[:], in0=cnt2[x][:, 0, :],
                        scalar1=float(CMAX - 1), scalar2=None, op0=ALU.min)
                    nc.vector.tensor_tensor(
                        out=a2[:], in0=a2[:], in1=valid2[:, q, :],
                        op=ALU.mult)
                    b2 = small.tile([128, NT], F32, tag="kb" + x, name="b2")
                    nc.vector.tensor_scalar(
                        out=b2[:], in0=cnt2[x][:, 1, :],
                        scalar1=float(CMAX - 1), scalar2=None, op0=ALU.min)
                    nc.vector.tensor_tensor(
                        out=b2[:], in0=b2[:], in1=valid2[:, q, :],
                        op=ALU.mult)
                    key2 = small.tile([128, NT], F32, tag="key" + x,
                                      name="key2")
                    nc.vector.scalar_tensor_tensor(
                        out=key2[:], in0=a2[:], scalar=float(CMAX * S),
                        in1=sn2[:, q, :], op0=ALU.mult, op1=ALU.add)
                    nc.vector.scalar_tensor_tensor(
                        out=key2[:], in0=b2[:], scalar=float(S), in1=key2[:],
                        op0=ALU.mult, op1=ALU.add)
                    keyi = small.tile([128, NT], I16, tag="keyi" + x,
                                      name="keyi")
                    nc.vector.tensor_copy(out=keyi[:], in_=key2[:])

                pend.append((b, x, keyi))

            # flush the last two rows individually so the end-of-kernel
            # serial chain (keys -> kscr -> wraps -> gather -> store) only
            # covers one row
            if b >= BPC - 2:
                flush(pend)
                pend = []
    nc.compile()
    return nc


def kernel(src_padded_nodes_neighbor_ids, dst_padded_nodes_neighbor_ids,
           src_padded_nodes_snapshots, dst_padded_nodes_snapshots,
           num_snapshots,
           agg_w1, agg_b1, agg_w2, agg_b2, enc_w1, enc_b1, enc_w2, enc_b2):
    import ml_dtypes

    tab = build_table(np.asarray(agg_w1), np.asarray(agg_b1),
                      np.asarray(agg_w2), np.asarray(agg_b2),
                      np.asarray(enc_w1), np.asarray(enc_b1),
                      np.asarray(enc_w2), np.asarray(enc_b2))

    if "nc" not in _NC_CACHE:
        _NC_CACHE["nc"] = build_nc()
    nc = _NC_CACHE["nc"]

    ids = {"s": np.asarray(src_padded_nodes_neighbor_ids).astype(np.int64),
           "d": np.asarray(dst_padded_nodes_neighbor_ids).astype(np.int64)}
    sn = {"s": np.asarray(src_padded_nodes_snapshots).astype(np.int64),
          "d": np.asarray(dst_padded_nodes_snapshots).astype(np.int64)}
    v = {x: ids[x] * 8 + (sn[x] - 1) for x in ("s", "d")}

    def digit_split(vz):
        """j-side and k-side bf16 digit-split operand blocks [8, n]."""
        vh = (vz >> 7).astype(np.float64)
        vl = (vz & 127).astype(np.float64)
        vh2, vl2 = vh * vh, vl * vl
        k2 = vh2 + vl2
        one = np.ones_like(vh)

        def s256(x):
            hi = np.floor(x / 256.0) * 256.0
            return hi, x - hi

        vh2hi, vh2lo = s256(vh2)
        vl2hi, vl2lo = s256(vl2)
        k2hi, k2lo = s256(k2)
        qj = np.stack([vh2hi, vh2lo, vh, vl2hi, vl2lo, vl, one, one], axis=1)
        qk = np.stack([one, one, -2.0 * vh, one, one, -2.0 * vl,
                       k2hi, k2lo], axis=1)
        return qj, qk

    in_maps = []
    for c in range(NCORES):
        sl = slice(c * BPC, (c + 1) * BPC)
        m = {"table": tab}
        # vsn[b, p, c(v/sn), x(s/d), t]
        vs = np.stack([np.stack([v["s"][sl], v["d"][sl]], axis=1),
                       np.stack([sn["s"][sl] - 1, sn["d"][sl] - 1], axis=1)],
                      axis=1).astype(np.float32)          # [BPC, 2, 2, L]
        vs = vs.reshape(-1, 2, 2, NT, 128).transpose(0, 4, 1, 2, 3)
        m["vsn"] = np.ascontiguousarray(vs)
        qj_d, qk_d = digit_split(v["d"][sl])
        _, qk_s = digit_split(v["s"][sl])
        q = np.concatenate([qj_d, qk_s, qk_d], axis=2)    # [BPC, 8, 3L]
        m["quint_d"] = np.ascontiguousarray(q.astype(ml_dtypes.bfloat16))
        for x in ("s", "d"):
            m[f"vi_{x}"] = np.ascontiguousarray(v[x][sl].astype(np.int16))
        in_maps.append(m)
    res = run_bass_kernel_spmd(nc, in_maps, core_ids=list(range(NCORES)),
                               trace=TRACE)
    LAST_RESULTS["res"] = res
    src_feat = np.concatenate([r["src_feat"] for r in res.results], axis=0)
    dst_feat = np.concatenate([r["dst_feat"] for r in res.results], axis=0)
    return (src_feat, dst_feat)
